# revision 10
# baseline (speedup 1.0000x reference)
"""Bass/Trainium2 kernel for nn_BidirectionalCrossAttentionGate.

Runs SPMD on 8 NeuronCores. Queries (rows of the two attention score
matrices) are sharded 1024/core; keys/values are replicated. All weight
algebra that does not depend on N is folded on the host:

  softmax(Q1 K1^T / 8) -> scores S1[n,m] = z1[n] . conv_x[m] with
      z1 = self_x_loc @ (q1_w^T k1_w)/8 + (k1_w^T q1_b)/8
  (row-constant terms dropped - softmax invariant; column-constant bias
   folded into z via the ones-row of the augmented x^T).

  V'' = [V | 1] so the PV matmul accumulates unnormalized outputs AND the
  softmax denominator in one PSUM group. Downstream only needs
  mean/max/dot(attn) per row, so normalization collapses to [1,1024] ops.

  The (N,4096) tensor-fusion einsum collapses to rank-9: gi1/gi2 are
  rank-3 in (mean, max, 1), so tensor = P8^T C8 with C8 (8,64) folded.
"""

import sys

if "/opt/trn_rl_repo" not in sys.path:
    sys.path.insert(0, "/opt/trn_rl_repo")

from contextlib import ExitStack

import numpy as np

import concourse.bass as bass
import concourse.tile as tile
from concourse import bacc, bass_isa, mybir
from concourse.bass_utils import run_bass_kernel_spmd

F32 = mybir.dt.float32
F32R = mybir.dt.float32r

N, D, NCORES = 8192, 64, 8
L = N // NCORES          # 1024 queries per core
KC = N // 128            # 64 key chunks
NB = L // 512            # 512-col blocks per core (2)

_cache = {}


def _r(ap):
    return ap.bitcast(F32R)


_INPUT_SPECS = [
    ("xsT", [65, N]), ("xcT", [65, N]),
    ("zq1", [65, L]), ("zq2", [65, L]),
    ("A1c", [65, D]), ("A2c", [65, D]),
    ("Vw1", [65, 65]), ("Vw2", [65, 65]),
    ("P1e3", [3, D]), ("P2e3", [3, D]),
    ("C8", [8, D]), ("btv", [D, 1]),
    ("WgHi", [128, 192]), ("WgLo", [64, 192]),
    ("bgHi", [128, 1]), ("bgLo", [64, 1]),
    ("wfHi", [128, 1]), ("wfLo", [64, 1]),
    ("mst1", [64, 2]), ("mst2", [64, 2]),
    ("bconst", [1, 1]),
]


def _build_nc():
    nc = bacc.Bacc("TRN2", target_bir_lowering=False, debug=False,
                   num_devices=NCORES)

    R_INPUTS = {"xsT", "xcT", "zq1", "zq2", "A1c", "A2c",
                "mst1", "mst2", "WgHi", "WgLo"}
    aps = {}
    for name, shape in _INPUT_SPECS:
        dt_in = F32R if name in R_INPUTS else F32
        aps[name] = nc.dram_tensor(name, shape, dt_in,
                                   kind="ExternalInput").ap()
    out = nc.dram_tensor("out", [1, L], F32, kind="ExternalOutput").ap()

    AF = mybir.ActivationFunctionType
    OP = mybir.AluOpType

    with tile.TileContext(nc) as tc, ExitStack() as ctx:
        const = ctx.enter_context(tc.tile_pool(name="const", bufs=1))
        work = ctx.enter_context(tc.tile_pool(name="work", bufs=1))
        vec = ctx.enter_context(tc.tile_pool(name="vec", bufs=1))
        espool = ctx.enter_context(tc.tile_pool(name="es", bufs=4))

        ct = {}
        for name, shape in _INPUT_SPECS:
            dt_in = F32R if name in R_INPUTS else F32
            t = const.tile(shape, dt_in, tag=name, name=f"t_{name}")
            nc.sync.dma_start(out=t[:], in_=aps[name])
            ct[name] = t
        one1_t = const.tile([1, 1], F32, tag="one1")
        nc.vector.memset(one1_t[:], 1.0)

        # --- z^T = Ac^T @ zq (c-bias via ones row of zq) -> SBUF [64, L] ---
        zT_ts = []
        with tc.tile_pool(name="zps_p", bufs=1, space="PSUM") as zpool:
            for d, (an, qn) in enumerate((("A1c", "zq1"), ("A2c", "zq2"))):
                zps = zpool.tile([64, L], F32, tag="zps", name=f"zps{d}")
                for b in range(NB):
                    nc.tensor.matmul(zps[:, bass.ts(b, 512)], ct[an][:],
                                     ct[qn][:, bass.ts(b, 512)],
                                     start=True, stop=True)
                # tag phA{d}: slot later reused by c0/c1
                zT = work.tile([64, L], F32R, tag=f"phA{d}", name=f"zT{d}")
                nc.vector.tensor_copy(zT[:], zps[:])
                zT_ts.append(zT)

        # --- V'' keys-major: [128, KC, 65]; col 64 of each chunk = 1 ---
        V_ts = []
        with tc.tile_pool(name="vps_p", bufs=2, space="PSUM") as vpool:
            for d, (vn, xn) in enumerate((("Vw1", "xcT"), ("Vw2", "xsT"))):
                Vt = const.tile([128, KC, 65], F32R, tag=f"Vt{d}",
                                name=f"Vt{d}")
                for g in range(KC // 4):
                    vps = vpool.tile([128, 4, 65], F32, tag="vps",
                                     name=f"vps{d}_{g}")
                    for j in range(4):
                        k = g * 4 + j
                        nc.tensor.matmul(
                            vps[:, j, :],
                            ct[xn][:, bass.ts(k, 128)].bitcast(F32),
                            ct[vn][:], start=True, stop=True)
                    nc.vector.tensor_copy(Vt[:, bass.ts(g, 4), :], vps[:])
                V_ts.append(Vt)

        # --- main attention loops ---
        stats = {}
        with tc.tile_pool(name="sps_p", bufs=2, space="PSUM") as spool, \
             tc.tile_pool(name="acc_p", bufs=1, space="PSUM") as apool:
            for d in range(2):
                keysT_t = ct["xcT"] if d == 0 else ct["xsT"]
                zT = zT_ts[d]
                Vt = V_ts[d]
                acc = apool.tile([65, L], F32, tag="acc", name=f"acc{d}")
                for k in range(KC):
                    sps = spool.tile([128, L], F32, tag="sps",
                                     name=f"sps{d}_{k}")
                    for b in range(NB):
                        nc.tensor.matmul(sps[:, bass.ts(b, 512)],
                                         keysT_t[0:64, bass.ts(k, 128)],
                                         zT[:, bass.ts(b, 512)],
                                         start=True, stop=True)
                    es = espool.tile([128, L], F32R, tag="es",
                                     name=f"es{d}_{k}")
                    nc.scalar.activation(es[:], sps[:], AF.Exp)
                    for b in range(NB):
                        nc.tensor.matmul(acc[:, bass.ts(b, 512)],
                                         Vt[:, k, :],
                                         es[:, bass.ts(b, 512)],
                                         start=(k == 0), stop=(k == KC - 1))

                # unnormalized outputs; denominator -> reciprocal directly
                # (acc[64:65] is a quadrant-aligned psum read)
                Ys = work.tile([64, L], F32R, tag=f"phB{d}", name=f"Ys{d}")
                nc.vector.tensor_copy(Ys[:], acc[0:64, :])
                rec = vec.tile([1, L], F32, tag=f"rec{d}", name=f"rec{d}")
                nc.vector.reciprocal(rec[:], acc[64:65, :])

                # max over partitions on gpsimd (engine otherwise idle)
                mxa = work.tile([64, L], F32, tag="mxa", bufs=2,
                                name=f"mxa{d}")
                nc.gpsimd.partition_all_reduce(mxa[:], Ys[:].bitcast(F32), 64,
                                               bass_isa.ReduceOp.max)
                mx_t = vec.tile([1, L], F32, tag=f"mx{d}", name=f"mx{d}")
                nc.vector.tensor_tensor(mx_t[:], mxa[0:1, :], rec[:], OP.mult)

                stats[d] = (Ys, rec, mx_t)

        # --- mean / wr-dot matmuls + normalization ---
        # matmul rhs tiles (operand base partition must match lhsT's = 0):
        # t8 = (m1m2, m1mx2, mx1m2, mx1mx2, m1, mx1, m2, mx2); u = (m, mx, 1)
        t8_t = work.tile([8, L], F32, tag="t8")
        u1_t = work.tile([3, L], F32, tag="u1")
        u2_t = work.tile([3, L], F32, tag="u2")
        nc.vector.memset(u1_t[:], 1.0)
        nc.vector.memset(u2_t[:], 1.0)

        norm = {}
        with tc.tile_pool(name="stat_p", bufs=1, space="PSUM") as statps:
            for d in range(2):
                Ys, rec, mx_t = stats[d]
                mst_t = ct["mst1"] if d == 0 else ct["mst2"]
                mean_ps = statps.tile([1, L], F32, tag=f"mean{d}",
                                      name=f"mean{d}")
                rp_ps = statps.tile([1, L], F32, tag=f"rpp{d}", name=f"rpp{d}")
                for b in range(NB):
                    bs = bass.ts(b, 512)
                    nc.tensor.matmul(mean_ps[:, bs], mst_t[:, 0:1],
                                     Ys[:, bs], start=True, stop=True)
                    nc.tensor.matmul(rp_ps[:, bs], mst_t[:, 1:2],
                                     Ys[:, bs], start=True, stop=True)
                m_t = vec.tile([1, L], F32, tag=f"m{d}", name=f"m{d}")
                nc.vector.tensor_tensor(m_t[:], mean_ps[:], rec[:], OP.mult)
                rd_t = vec.tile([1, L], F32, tag=f"rd{d}", name=f"rd{d}")
                nc.vector.tensor_tensor(rd_t[:], rp_ps[:], rec[:], OP.mult)
                norm[d] = (m_t, rd_t)

        m1_t, rd1_t = norm[0]
        m2_t, rd2_t = norm[1]
        mx1_t = stats[0][2]
        mx2_t = stats[1][2]

        for i, (a_t, b_t) in enumerate(((m1_t, m2_t), (m1_t, mx2_t),
                                        (mx1_t, m2_t), (mx1_t, mx2_t))):
            pp = vec.tile([1, L], F32, tag="pp", bufs=2, name=f"pp{i}")
            nc.vector.tensor_tensor(pp[:], a_t[:], b_t[:], OP.mult)
            nc.sync.dma_start(out=t8_t[i:i + 1, :], in_=pp[:])

        for dst, row, st in ((t8_t, 4, m1_t), (t8_t, 5, mx1_t),
                             (t8_t, 6, m2_t), (t8_t, 7, mx2_t),
                             (u1_t, 0, m1_t), (u1_t, 1, mx1_t),
                             (u2_t, 0, m2_t), (u2_t, 1, mx2_t)):
            nc.sync.dma_start(out=dst[row:row + 1, :], in_=st[:])

        # --- gi1/gi2 (-> gps), tensor (-> tps), gate, output ---
        with tc.tile_pool(name="post_p", bufs=1, space="PSUM") as post, \
             tc.tile_pool(name="yp_p", bufs=1, space="PSUM") as ypool:
            gps = post.tile([128, L], F32, tag="post", name="gps")
            tps = post.tile([64, L], F32, tag="post2", name="tps")
            for b in range(NB):
                bs = bass.ts(b, 512)
                nc.tensor.matmul(gps[0:64, bs], ct["P1e3"][:],
                                 u1_t[:, bs], start=True, stop=True)
                nc.tensor.matmul(gps[64:128, bs], ct["P2e3"][:],
                                 u2_t[:, bs], start=True, stop=True)
                nc.tensor.matmul(tps[:, bs], ct["C8"][:],
                                 t8_t[:, bs], start=True, stop=True)

            c0 = work.tile([128, L], F32R, tag="phA0", name="c0")
            nc.vector.tensor_copy(c0[:], gps[:])
            c1 = work.tile([64, L], F32R, tag="phA1", name="c1")
            # tensor = gelu(tps + bt)  (exact erf gelu)
            nc.scalar.activation(c1[:], tps[:], AF.Gelu, bias=ct["btv"][:])

            g0ps = post.tile([128, L], F32, tag="post", name="g0ps")
            g1ps = post.tile([64, L], F32, tag="post2", name="g1ps")
            for b in range(NB):
                bs = bass.ts(b, 512)
                nc.tensor.matmul(g0ps[:, bs], ct["WgHi"][:, 0:128],
                                 c0[:, bs], start=True, stop=False)
                nc.tensor.matmul(g0ps[:, bs], ct["WgLo"][:, 0:128],
                                 c1[:, bs], start=False, stop=True)
                nc.tensor.matmul(g1ps[:, bs], ct["WgHi"][:, 128:192],
                                 c0[:, bs], start=True, stop=False)
                nc.tensor.matmul(g1ps[:, bs], ct["WgLo"][:, 128:192],
                                 c1[:, bs], start=False, stop=True)

            g0 = work.tile([128, L], F32, tag="phB0", name="g0")
            nc.scalar.activation(g0[:], g0ps[:], AF.Sigmoid, bias=ct["bgHi"][:])
            g1 = work.tile([64, L], F32, tag="phB1", name="g1")
            nc.scalar.activation(g1[:], g1ps[:], AF.Sigmoid, bias=ct["bgLo"][:])

            gc0 = espool.tile([128, L], F32, tag="es", name="gc0")
            nc.vector.tensor_tensor(gc0[:], g0[:], c0[:].bitcast(F32),
                                    OP.mult)
            gc1 = espool.tile([64, L], F32, tag="es", name="gc1")
            nc.vector.tensor_tensor(gc1[:], g1[:], c1[:].bitcast(F32),
                                    OP.mult)

            # y = sigmoid(wf.gc + rd1 + rd2 + bconst)
            yp = ypool.tile([1, L], F32, tag="yp")
            for b in range(NB):
                bs = bass.ts(b, 512)
                nc.tensor.matmul(yp[:, bs], ct["wfHi"][:], gc0[:, bs],
                                 start=True, stop=False)
                nc.tensor.matmul(yp[:, bs], ct["wfLo"][:], gc1[:, bs],
                                 start=False, stop=False)
                nc.tensor.matmul(yp[:, bs], one1_t[:], rd1_t[:, bs],
                                 start=False, stop=False)
                nc.tensor.matmul(yp[:, bs], one1_t[:], rd2_t[:, bs],
                                 start=False, stop=True)

            y = work.tile([1, L], F32, tag="y")
            nc.scalar.activation(y[:], yp[:], AF.Sigmoid, bias=ct["bconst"][:])
            nc.sync.dma_start(out=out, in_=y[:])

    nc.compile()
    return nc


def _fold_params(p):
    f32 = np.float32
    A1 = (p["q1_w"].T @ p["k1_w"]) / 8.0
    c1 = (p["k1_w"].T @ p["q1_b"]) / 8.0
    A2 = (p["q2_w"].T @ p["k2_w"]) / 8.0
    c2 = (p["k2_w"].T @ p["q2_b"]) / 8.0
    A1c = np.concatenate([A1, c1[None, :]], 0).astype(f32)    # (65, 64)
    A2c = np.concatenate([A2, c2[None, :]], 0).astype(f32)

    def vaug(w, b):
        M = np.zeros((65, 65), f32)
        M[:64, :64] = w.T
        M[64, :64] = b
        M[64, 64] = 1.0
        return M

    Vw1 = vaug(p["v1_w"], p["v1_b"])
    Vw2 = vaug(p["v2_w"], p["v2_b"])

    P1e = np.concatenate([p["pool_fc1_w"], p["pool_fc1_b"][:, None]], 1)
    P2e = np.concatenate([p["pool_fc2_w"], p["pool_fc2_b"][:, None]], 1)
    Wt = p["tensor_fc_w"].reshape(D, D, D)
    C = np.einsum("ia,jb,kij->abk", P1e, P2e, Wt)
    bt = (p["tensor_fc_b"] + C[2, 2]).astype(f32)
    C8 = np.stack([C[0, 0], C[0, 1], C[1, 0], C[1, 1],
                   C[0, 2], C[1, 2], C[2, 0], C[2, 1]], 0).astype(f32)

    WgT = np.ascontiguousarray(p["gate_w"].T).astype(f32)      # (192,192)
    w_o = p["output_w"][0]
    wf = (p["fusion_w"].T @ w_o).astype(f32)                   # (192,)
    wr = (p["res_transform_w"].T @ w_o).astype(f32)            # (128,)
    bconst = np.asarray(
        p["fusion_b"] @ w_o + p["res_transform_b"] @ w_o + p["output_b"][0],
        f32).reshape(1, 1)

    mcol = np.full((64, 1), 1.0 / 64.0, f32)
    mst1 = np.concatenate([mcol, wr[:64, None]], 1).astype(f32)
    mst2 = np.concatenate([mcol, wr[64:, None]], 1).astype(f32)

    return {
        "A1c": A1c, "A2c": A2c, "Vw1": Vw1, "Vw2": Vw2,
        "P1e3": np.ascontiguousarray(P1e.T).astype(f32),
        "P2e3": np.ascontiguousarray(P2e.T).astype(f32),
        "C8": C8, "btv": bt[:, None],
        "WgHi": np.ascontiguousarray(WgT[0:128]),
        "WgLo": np.ascontiguousarray(WgT[128:192]),
        "bgHi": p["gate_b"][:128, None].astype(f32),
        "bgLo": p["gate_b"][128:, None].astype(f32),
        "wfHi": wf[:128, None], "wfLo": wf[128:, None],
        "mst1": mst1, "mst2": mst2, "bconst": bconst,
    }


def make_in_maps(self_x, conv_x, params):
    self_x = np.ascontiguousarray(np.asarray(self_x, np.float32))
    conv_x = np.ascontiguousarray(np.asarray(conv_x, np.float32))
    p = {k: np.asarray(v, np.float32) for k, v in params.items()}

    folded = _fold_params(p)
    ones = np.ones((1, N), np.float32)
    xsT = np.ascontiguousarray(np.concatenate([self_x.T, ones], 0))
    xcT = np.ascontiguousarray(np.concatenate([conv_x.T, ones], 0))
    onesL = np.ones((1, L), np.float32)

    in_maps = []
    for c in range(NCORES):
        sl = slice(c * L, (c + 1) * L)
        m = dict(folded)
        m["xsT"] = xsT
        m["xcT"] = xcT
        m["zq1"] = np.ascontiguousarray(
            np.concatenate([self_x[sl].T, onesL], 0))
        m["zq2"] = np.ascontiguousarray(
            np.concatenate([conv_x[sl].T, onesL], 0))
        in_maps.append(m)
    return in_maps


def kernel(self_x, conv_x, params):
    if "nc" not in _cache:
        _cache["nc"] = _build_nc()
    nc = _cache["nc"]

    in_maps = make_in_maps(self_x, conv_x, params)
    res = run_bass_kernel_spmd(nc, in_maps, list(range(NCORES)))
    out = np.concatenate([res.results[c]["out"].reshape(L)
                          for c in range(NCORES)])
    return out.reshape(N, 1).astype(np.float32)


# revision 12
# speedup vs baseline: 1.2619x; 1.2619x over previous
"""Bass/Trainium2 kernel for nn_BidirectionalCrossAttentionGate.

Runs SPMD on 8 NeuronCores. Queries (rows of the two attention score
matrices) are sharded 1024/core; keys/values are replicated. All weight
algebra that does not depend on N is folded on the host:

  softmax(Q1 K1^T / 8) -> scores S1[n,m] = z1[n] . conv_x[m] with
      z1 = self_x_loc @ (q1_w^T k1_w)/8 + (k1_w^T q1_b)/8
  (row-constant terms dropped - softmax invariant; column-constant bias
   folded into z via the ones-row of the augmented x^T).

  V'' = [V | 1] so the PV matmul accumulates unnormalized outputs AND the
  softmax denominator in one PSUM group. Downstream only needs
  mean/max/dot(attn) per row, so normalization collapses to [1,1024] ops.

  The (N,4096) tensor-fusion einsum collapses to rank-9: gi1/gi2 are
  rank-3 in (mean, max, 1), so tensor = P8^T C8 with C8 (8,64) folded.
"""

import sys

if "/opt/trn_rl_repo" not in sys.path:
    sys.path.insert(0, "/opt/trn_rl_repo")

from contextlib import ExitStack

import ml_dtypes
import numpy as np

import concourse.bass as bass
import concourse.tile as tile
from concourse import bacc, bass_isa, mybir
from concourse.bass_utils import run_bass_kernel_spmd

F32 = mybir.dt.float32
F32R = mybir.dt.float32r
BF16 = mybir.dt.bfloat16

N, D, NCORES = 8192, 64, 8
L = N // NCORES          # 1024 queries per core
KC = N // 128            # 64 key chunks
NB = L // 512            # 512-col blocks per core (2)

_cache = {}


def _r(ap):
    return ap.bitcast(F32R)


_INPUT_SPECS = [
    ("xsT", [65, N]), ("xcT", [65, N]),
    ("xsTb", [65, N]), ("xcTb", [65, N]),
    ("zq1", [65, L]), ("zq2", [65, L]),
    ("A1c", [65, D]), ("A2c", [65, D]),
    ("Vw1", [65, 66]), ("Vw2", [65, 66]),
    ("P1e3", [3, D]), ("P2e3", [3, D]),
    ("C8", [8, D]), ("btv", [D, 1]),
    ("WgHi", [128, 192]), ("WgLo", [64, 192]),
    ("bgHi", [128, 1]), ("bgLo", [64, 1]),
    ("wfHi", [128, 1]), ("wfLo", [64, 1]),
    ("mst1", [64, 2]), ("mst2", [64, 2]),
    ("bconst", [1, 1]),
]


def _build_nc():
    nc = bacc.Bacc("TRN2", target_bir_lowering=False, debug=False,
                   num_devices=NCORES)

    R_INPUTS = {"xsT", "xcT", "zq1", "zq2", "A1c", "A2c",
                "mst1", "mst2", "WgHi", "WgLo"}
    B_INPUTS = {"xsTb", "xcTb", "Vw1", "Vw2", "wfHi", "wfLo"}
    aps = {}
    for name, shape in _INPUT_SPECS:
        dt_in = (F32R if name in R_INPUTS
                 else BF16 if name in B_INPUTS else F32)
        aps[name] = nc.dram_tensor(name, shape, dt_in,
                                   kind="ExternalInput").ap()
    out = nc.dram_tensor("out", [1, L], F32, kind="ExternalOutput").ap()

    AF = mybir.ActivationFunctionType
    OP = mybir.AluOpType

    with tile.TileContext(nc) as tc, ExitStack() as ctx:
        const = ctx.enter_context(tc.tile_pool(name="const", bufs=1))
        work = ctx.enter_context(tc.tile_pool(name="work", bufs=1))
        vec = ctx.enter_context(tc.tile_pool(name="vec", bufs=1))
        espool = ctx.enter_context(tc.tile_pool(name="es", bufs=4))

        ct = {}
        for name, shape in _INPUT_SPECS:
            dt_in = (F32R if name in R_INPUTS
                     else BF16 if name in B_INPUTS else F32)
            tag = {"zq1": "phB0", "zq2": "phB1"}.get(name, name)
            pool_ = work if name in ("zq1", "zq2") else const
            t = pool_.tile(shape, dt_in, tag=tag, name=f"t_{name}")
            nc.sync.dma_start(out=t[:], in_=aps[name])
            ct[name] = t
        one1_t = const.tile([1, 1], F32, tag="one1")
        nc.vector.memset(one1_t[:], 1.0)

        # --- z^T = Ac^T @ zq (c-bias via ones row of zq) -> SBUF [64, L] ---
        zT_ts = []
        with tc.tile_pool(name="zps_p", bufs=1, space="PSUM") as zpool:
            for d, (an, qn) in enumerate((("A1c", "zq1"), ("A2c", "zq2"))):
                zps = zpool.tile([64, L], F32, tag="zps", name=f"zps{d}")
                for b in range(NB):
                    nc.tensor.matmul(zps[:, bass.ts(b, 512)], ct[an][:],
                                     ct[qn][:, bass.ts(b, 512)],
                                     start=True, stop=True)
                # tag phA{d}: slot later reused by c0/c1
                zT = work.tile([64, L], F32R, tag=f"phA{d}", name=f"zT{d}")
                nc.vector.tensor_copy(zT[:], zps[:])
                zT_ts.append(zT)

        # --- V'' keys-major: [128, KC, 65]; col 64 of each chunk = 1 ---
        V_ts = []
        with tc.tile_pool(name="vps_p", bufs=2, space="PSUM") as vpool:
            for d, (vn, xn) in enumerate((("Vw1", "xcTb"), ("Vw2", "xsTb"))):
                Vt = const.tile([128, KC, 66], BF16, tag=f"Vt{d}",
                                name=f"Vt{d}")
                for g in range(KC // 4):
                    vps = vpool.tile([128, 4, 66], F32, tag="vps",
                                     name=f"vps{d}_{g}")
                    for j in range(4):
                        k = g * 4 + j
                        nc.tensor.matmul(
                            vps[:, j, :],
                            ct[xn][:, bass.ts(k, 128)],
                            ct[vn][:], start=True, stop=True)
                    nc.vector.tensor_copy(Vt[:, bass.ts(g, 4), :], vps[:])
                V_ts.append(Vt)

        # --- main attention loops ---
        stats = {}
        with tc.tile_pool(name="sps_p", bufs=2, space="PSUM") as spool, \
             tc.tile_pool(name="acc_p", bufs=1, space="PSUM") as apool:
            for d in range(2):
                keysT_t = ct["xcT"] if d == 0 else ct["xsT"]
                zT = zT_ts[d]
                Vt = V_ts[d]
                acc = apool.tile([66, L], F32, tag="acc", name=f"acc{d}")
                for k in range(KC):
                    sps = spool.tile([128, L], F32, tag="sps",
                                     name=f"sps{d}_{k}")
                    for b in range(NB):
                        nc.tensor.matmul(sps[:, bass.ts(b, 512)],
                                         keysT_t[0:64, bass.ts(k, 128)],
                                         zT[:, bass.ts(b, 512)],
                                         start=True, stop=True)
                    es = espool.tile([128, L], BF16, tag="es",
                                     name=f"es{d}_{k}")
                    nc.scalar.activation(es[:], sps[:], AF.Exp)
                    for b in range(NB):
                        nc.tensor.matmul(acc[:, bass.ts(b, 512)],
                                         Vt[:, k, :],
                                         es[:, bass.ts(b, 512)],
                                         start=(k == 0), stop=(k == KC - 1))

                # unnormalized outputs; denominator -> reciprocal directly
                # (acc[64:65] is a quadrant-aligned psum read)
                Ys = work.tile([64, L], F32R, tag=f"phB{d}", name=f"Ys{d}")
                nc.vector.tensor_copy(Ys[:], acc[0:64, :])
                rec = vec.tile([1, L], F32, tag=f"rec{d}", name=f"rec{d}")
                nc.vector.reciprocal(rec[:], acc[64:65, :])

                # max over partitions on gpsimd (engine otherwise idle)
                mxa = work.tile([64, L], F32, tag="mxa", bufs=1,
                                name=f"mxa{d}")
                nc.gpsimd.partition_all_reduce(mxa[:], Ys[:].bitcast(F32), 64,
                                               bass_isa.ReduceOp.max)
                mx_t = vec.tile([1, L], F32, tag=f"mx{d}", name=f"mx{d}")
                nc.vector.tensor_tensor(mx_t[:], mxa[0:1, :], rec[:], OP.mult)

                stats[d] = (Ys, rec, mx_t)

        # --- mean / wr-dot matmuls + normalization ---
        # matmul rhs tiles (operand base partition must match lhsT's = 0):
        # t8 = (m1m2, m1mx2, mx1m2, mx1mx2, m1, mx1, m2, mx2); u = (m, mx, 1)
        t8_t = work.tile([8, L], F32, tag="t8")
        u1_t = work.tile([3, L], F32, tag="u1")
        u2_t = work.tile([3, L], F32, tag="u2")
        nc.vector.memset(u1_t[:], 1.0)
        nc.vector.memset(u2_t[:], 1.0)

        norm = {}
        with tc.tile_pool(name="stat_p", bufs=1, space="PSUM") as statps:
            for d in range(2):
                Ys, rec, mx_t = stats[d]
                mst_t = ct["mst1"] if d == 0 else ct["mst2"]
                mean_ps = statps.tile([1, L], F32, tag=f"mean{d}",
                                      name=f"mean{d}")
                rp_ps = statps.tile([1, L], F32, tag=f"rpp{d}", name=f"rpp{d}")
                for b in range(NB):
                    bs = bass.ts(b, 512)
                    nc.tensor.matmul(mean_ps[:, bs], mst_t[:, 0:1],
                                     Ys[:, bs], start=True, stop=True)
                    nc.tensor.matmul(rp_ps[:, bs], mst_t[:, 1:2],
                                     Ys[:, bs], start=True, stop=True)
                m_t = vec.tile([1, L], F32, tag=f"m{d}", name=f"m{d}")
                nc.vector.tensor_tensor(m_t[:], mean_ps[:], rec[:], OP.mult)
                rd_t = vec.tile([1, L], F32, tag=f"rd{d}", name=f"rd{d}")
                nc.vector.tensor_tensor(rd_t[:], rp_ps[:], rec[:], OP.mult)
                norm[d] = (m_t, rd_t)

        m1_t, rd1_t = norm[0]
        m2_t, rd2_t = norm[1]
        mx1_t = stats[0][2]
        mx2_t = stats[1][2]

        for i, (a_t, b_t) in enumerate(((m1_t, m2_t), (m1_t, mx2_t),
                                        (mx1_t, m2_t), (mx1_t, mx2_t))):
            pp = vec.tile([1, L], F32, tag="pp", bufs=2, name=f"pp{i}")
            nc.vector.tensor_tensor(pp[:], a_t[:], b_t[:], OP.mult)
            nc.sync.dma_start(out=t8_t[i:i + 1, :], in_=pp[:])

        for dst, row, st in ((t8_t, 4, m1_t), (t8_t, 5, mx1_t),
                             (t8_t, 6, m2_t), (t8_t, 7, mx2_t),
                             (u1_t, 0, m1_t), (u1_t, 1, mx1_t),
                             (u2_t, 0, m2_t), (u2_t, 1, mx2_t)):
            nc.sync.dma_start(out=dst[row:row + 1, :], in_=st[:])

        # --- gi1/gi2 (-> gps), tensor (-> tps), gate, output ---
        with tc.tile_pool(name="post_p", bufs=1, space="PSUM") as post, \
             tc.tile_pool(name="yp_p", bufs=1, space="PSUM") as ypool:
            gps = post.tile([128, L], F32, tag="post", name="gps")
            tps = post.tile([64, L], F32, tag="post2", name="tps")
            for b in range(NB):
                bs = bass.ts(b, 512)
                nc.tensor.matmul(gps[0:64, bs], ct["P1e3"][:],
                                 u1_t[:, bs], start=True, stop=True)
                nc.tensor.matmul(gps[64:128, bs], ct["P2e3"][:],
                                 u2_t[:, bs], start=True, stop=True)
                nc.tensor.matmul(tps[:, bs], ct["C8"][:],
                                 t8_t[:, bs], start=True, stop=True)

            c0 = work.tile([128, L], F32R, tag="phA0", name="c0")
            nc.vector.tensor_copy(c0[:], gps[:])
            c1 = work.tile([64, L], F32R, tag="phA1", name="c1")
            # tensor = gelu(tps + bt)  (exact erf gelu)
            nc.scalar.activation(c1[:], tps[:], AF.Gelu, bias=ct["btv"][:])

            g0ps = post.tile([128, L], F32, tag="post", name="g0ps")
            g1ps = post.tile([64, L], F32, tag="post2", name="g1ps")
            for b in range(NB):
                bs = bass.ts(b, 512)
                nc.tensor.matmul(g0ps[:, bs], ct["WgHi"][:, 0:128],
                                 c0[:, bs], start=True, stop=False)
                nc.tensor.matmul(g0ps[:, bs], ct["WgLo"][:, 0:128],
                                 c1[:, bs], start=False, stop=True)
                nc.tensor.matmul(g1ps[:, bs], ct["WgHi"][:, 128:192],
                                 c0[:, bs], start=True, stop=False)
                nc.tensor.matmul(g1ps[:, bs], ct["WgLo"][:, 128:192],
                                 c1[:, bs], start=False, stop=True)

            g0 = work.tile([128, L], F32, tag="phB0", name="g0")
            nc.scalar.activation(g0[:], g0ps[:], AF.Sigmoid, bias=ct["bgHi"][:])
            g1 = work.tile([64, L], F32, tag="phB1", name="g1")
            nc.scalar.activation(g1[:], g1ps[:], AF.Sigmoid, bias=ct["bgLo"][:])

            gc0 = espool.tile([128, L], BF16, tag="es", name="gc0")
            nc.vector.tensor_tensor(gc0[:], g0[:], c0[:].bitcast(F32),
                                    OP.mult)
            gc1 = espool.tile([64, L], BF16, tag="es", name="gc1")
            nc.vector.tensor_tensor(gc1[:], g1[:], c1[:].bitcast(F32),
                                    OP.mult)

            # y = sigmoid(wf.gc + rd1 + rd2 + bconst)
            yp = ypool.tile([1, L], F32, tag="yp")
            for b in range(NB):
                bs = bass.ts(b, 512)
                nc.tensor.matmul(yp[:, bs], ct["wfHi"][:], gc0[:, bs],
                                 start=True, stop=False)
                nc.tensor.matmul(yp[:, bs], ct["wfLo"][:], gc1[:, bs],
                                 start=False, stop=False)
                nc.tensor.matmul(yp[:, bs], one1_t[:], rd1_t[:, bs],
                                 start=False, stop=False)
                nc.tensor.matmul(yp[:, bs], one1_t[:], rd2_t[:, bs],
                                 start=False, stop=True)

            y = work.tile([1, L], F32, tag="y")
            nc.scalar.activation(y[:], yp[:], AF.Sigmoid, bias=ct["bconst"][:])
            nc.sync.dma_start(out=out, in_=y[:])

    nc.compile()
    return nc


def _fold_params(p):
    f32 = np.float32
    A1 = (p["q1_w"].T @ p["k1_w"]) / 8.0
    c1 = (p["k1_w"].T @ p["q1_b"]) / 8.0
    A2 = (p["q2_w"].T @ p["k2_w"]) / 8.0
    c2 = (p["k2_w"].T @ p["q2_b"]) / 8.0
    A1c = np.concatenate([A1, c1[None, :]], 0).astype(f32)    # (65, 64)
    A2c = np.concatenate([A2, c2[None, :]], 0).astype(f32)

    def vaug(w, b):
        M = np.zeros((65, 66), f32)
        M[:64, :64] = w.T
        M[64, :64] = b
        M[64, 64] = 1.0
        return M.astype(ml_dtypes.bfloat16)

    Vw1 = vaug(p["v1_w"], p["v1_b"])
    Vw2 = vaug(p["v2_w"], p["v2_b"])

    P1e = np.concatenate([p["pool_fc1_w"], p["pool_fc1_b"][:, None]], 1)
    P2e = np.concatenate([p["pool_fc2_w"], p["pool_fc2_b"][:, None]], 1)
    Wt = p["tensor_fc_w"].reshape(D, D, D)
    C = np.einsum("ia,jb,kij->abk", P1e, P2e, Wt)
    bt = (p["tensor_fc_b"] + C[2, 2]).astype(f32)
    C8 = np.stack([C[0, 0], C[0, 1], C[1, 0], C[1, 1],
                   C[0, 2], C[1, 2], C[2, 0], C[2, 1]], 0).astype(f32)

    WgT = np.ascontiguousarray(p["gate_w"].T).astype(f32)      # (192,192)
    w_o = p["output_w"][0]
    wf = (p["fusion_w"].T @ w_o).astype(f32)                   # (192,)
    wr = (p["res_transform_w"].T @ w_o).astype(f32)            # (128,)
    bconst = np.asarray(
        p["fusion_b"] @ w_o + p["res_transform_b"] @ w_o + p["output_b"][0],
        f32).reshape(1, 1)

    mcol = np.full((64, 1), 1.0 / 64.0, f32)
    mst1 = np.concatenate([mcol, wr[:64, None]], 1).astype(f32)
    mst2 = np.concatenate([mcol, wr[64:, None]], 1).astype(f32)

    return {
        "A1c": A1c, "A2c": A2c, "Vw1": Vw1, "Vw2": Vw2,
        "P1e3": np.ascontiguousarray(P1e.T).astype(f32),
        "P2e3": np.ascontiguousarray(P2e.T).astype(f32),
        "C8": C8, "btv": bt[:, None],
        "WgHi": np.ascontiguousarray(WgT[0:128]),
        "WgLo": np.ascontiguousarray(WgT[128:192]),
        "bgHi": p["gate_b"][:128, None].astype(f32),
        "bgLo": p["gate_b"][128:, None].astype(f32),
        "wfHi": wf[:128, None].astype(ml_dtypes.bfloat16),
        "wfLo": wf[128:, None].astype(ml_dtypes.bfloat16),
        "mst1": mst1, "mst2": mst2, "bconst": bconst,
    }


def make_in_maps(self_x, conv_x, params):
    self_x = np.ascontiguousarray(np.asarray(self_x, np.float32))
    conv_x = np.ascontiguousarray(np.asarray(conv_x, np.float32))
    p = {k: np.asarray(v, np.float32) for k, v in params.items()}

    folded = _fold_params(p)
    ones = np.ones((1, N), np.float32)
    xsT = np.ascontiguousarray(np.concatenate([self_x.T, ones], 0))
    xcT = np.ascontiguousarray(np.concatenate([conv_x.T, ones], 0))
    xsTb = xsT.astype(ml_dtypes.bfloat16)
    xcTb = xcT.astype(ml_dtypes.bfloat16)
    onesL = np.ones((1, L), np.float32)

    in_maps = []
    for c in range(NCORES):
        sl = slice(c * L, (c + 1) * L)
        m = dict(folded)
        m["xsT"] = xsT
        m["xcT"] = xcT
        m["xsTb"] = xsTb
        m["xcTb"] = xcTb
        m["zq1"] = np.ascontiguousarray(
            np.concatenate([self_x[sl].T, onesL], 0))
        m["zq2"] = np.ascontiguousarray(
            np.concatenate([conv_x[sl].T, onesL], 0))
        in_maps.append(m)
    return in_maps


def kernel(self_x, conv_x, params):
    if "nc" not in _cache:
        _cache["nc"] = _build_nc()
    nc = _cache["nc"]

    in_maps = make_in_maps(self_x, conv_x, params)
    res = run_bass_kernel_spmd(nc, in_maps, list(range(NCORES)))
    out = np.concatenate([res.results[c]["out"].reshape(L)
                          for c in range(NCORES)])
    return out.reshape(N, 1).astype(np.float32)
